# revision 4
# baseline (speedup 1.0000x reference)
"""LAHGCN hypergraph-conv kernel for 8 Trainium2 NeuronCores — v5.

v5 vs v4a:
  - All per-incidence gathers use indirect_dma_start (hardware dynamic DGE,
    int32 per-partition row offsets) instead of dma_gather (software Q7
    descriptor generation at ~8ns/row, which was the B/C bottleneck).
  - int32 offsets remove the int16 row-index limit: B and phase-D-overflow
    lo/hi stream splits are merged into single streams.
  - Phases D/E stay dense-block (no per-incidence work at all).
"""
import numpy as np

N, E, NNZ = 50000, 20000, 1600000
CONCAT, C_IN, C_HID = 4, 256, 256
C = CONCAT * C_HID            # 1024
C_OUT, C_OUT_P = 40, 64
CO = 64                       # padded col count for smooth-2 tables (bf16)
W = 8
NPC_R, EPC_R = N // W, E // W           # 6250, 2500 real per core
NBLK, EBLK = 49, 20
NPC, EPC = NBLK * 128, EBLK * 128       # 6272, 2560 padded per core
NP_, EP_ = W * NPC, W * EPC             # 50176, 20480
NBLK_F, EBLK_F = NP_ // 128, EP_ // 128  # 392, 160 full-table blocks
BATCH = 8                               # one-hot chunks per is_equal build
ESW = [(0, 49)]                         # phase E psum sweep over node blocks


def _streams32(rows, segpos, nblk, K):
    """int32 idx table [128, nblk*K] + seg table [128, nblk*K].
    Entry i of block b -> partition i%128, column b*K + i//128."""
    idx = np.zeros((128, nblk * K), np.int32)
    seg = np.full((128, nblk * K), -1.0, np.float32)
    for b in range(nblk):
        r, p = rows[b], segpos[b]
        n = len(r)
        assert n <= K * 128
        part = np.arange(n) % 128
        col = b * K + np.arange(n) // 128
        idx[part, col] = r.astype(np.int32)
        seg[part, col] = p.astype(np.float32)
    return idx, seg


def _split_rank(src_loc, pair, pos, npairs):
    """Rank entries within (src row mod 128, pair); k=0 goes to the dense seg
    table [128, npairs], k>0 to overflow lists."""
    order = np.lexsort((pos, pair * 128 + src_loc))
    sl, pr, po = src_loc[order], pair[order], pos[order]
    key = pr * 128 + sl
    first = np.ones(len(key), bool)
    first[1:] = key[1:] != key[:-1]
    seg = np.full((128, npairs), -1.0, np.float32)
    seg[sl[first], pr[first]] = po[first].astype(np.float32)
    return seg, order[~first]


def _prep(node_idx, edge_idx, dv_is, de_inv):
    """All host-side index prep. Returns per-core dicts of arrays + loop Ks."""
    nrow = (node_idx // NPC_R) * NPC + node_idx % NPC_R    # node -> y row
    erow = (edge_idx // EPC_R) * EPC + edge_idx % EPC_R    # edge -> ef row
    p1 = np.argsort(edge_idx, kind="stable")
    e1, n1 = edge_idx[p1], nrow[p1]
    p2 = np.argsort(node_idx, kind="stable")
    n2, e2 = node_idx[p2], erow[p2]
    per = []
    for c in range(W):
        m1 = (e1 >= c * EPC_R) & (e1 < (c + 1) * EPC_R)
        el = e1[m1] - c * EPC_R        # local edge row (edge-sorted)
        nr = n1[m1]                    # full node row (y table)
        ab_rows, ab_pos = [], []
        for b in range(EBLK):
            mb = (el >= b * 128) & (el < (b + 1) * 128)
            ab_rows.append(nr[mb]); ab_pos.append(el[mb] - b * 128)
        m2 = (n2 >= c * NPC_R) & (n2 < (c + 1) * NPC_R)
        nl = n2[m2] - c * NPC_R        # local node row (node-sorted)
        er = e2[m2]                    # full edge row (ef table)
        c_rows, c_pos = [], []
        for b in range(NBLK):
            mb = (nl >= b * 128) & (nl < (b + 1) * 128)
            c_rows.append(er[mb]); c_pos.append(nl[mb] - b * 128)

        # phase D dense/overflow (edge-sorted): pair (nb 0..391, eb 0..19)
        nbD, ebD = nr // 128, el // 128
        pairD = nbD * EBLK + ebD
        segD, ovD = _split_rank(nr % 128, pairD, el % 128, NBLK_F * EBLK)
        ovD_r, ovD_eb, ovD_pos = nr[ovD], ebD[ovD], (el % 128)[ovD]
        d_rows, d_pos = [], []
        for b in range(EBLK):
            mb = ovD_eb == b
            d_rows.append(ovD_r[mb]); d_pos.append(ovD_pos[mb])

        # phase E dense/overflow (node-sorted): pair (eb 0..159, nb 0..48)
        ebE, nbE = er // 128, nl // 128
        pairE = ebE * NBLK + nbE
        segE, ovE = _split_rank(er % 128, pairE, nl % 128, EBLK_F * NBLK)
        ovE_r, ovE_nb, ovE_pos = er[ovE], nbE[ovE], (nl % 128)[ovE]
        e_rows, e_pos = [], []
        for b in range(NBLK):
            mb = ovE_nb == b
            e_rows.append(ovE_r[mb]); e_pos.append(ovE_pos[mb])

        per.append((ab_rows, ab_pos, c_rows, c_pos,
                    segD, (d_rows, d_pos), segE, (e_rows, e_pos)))

    kmax = lambda lists: max(1, max(max(((len(r) + 127) // 128 for r in p),
                                        default=0) for p in lists))
    KAB = kmax([p[0] for p in per])
    KC = kmax([p[2] for p in per])
    KOD = kmax([p[5][0] for p in per])
    KOE = kmax([p[7][0] for p in per])
    cores = []
    for c in range(W):
        (ab_rows, ab_pos, c_rows, c_pos,
         segD, (d_r, d_p), segE, (e_r, e_p)) = per[c]
        iAB, sAB = _streams32(ab_rows, ab_pos, EBLK, KAB)
        iC, sC = _streams32(c_rows, c_pos, NBLK, KC)
        iOD, sOD = _streams32(d_r, d_p, EBLK, KOD)
        iOE, sOE = _streams32(e_r, e_p, NBLK, KOE)
        dv = np.zeros(NPC, np.float32)
        dv[:NPC_R] = dv_is[c * NPC_R:(c + 1) * NPC_R]
        de = np.zeros(EPC, np.float32)
        de[:EPC_R] = de_inv[c * EPC_R:(c + 1) * EPC_R]
        cores.append(dict(
            idxAB=iAB, segAB=sAB, idxC=iC, segC=sC,
            idxOD=iOD, segOD=sOD, idxOE=iOE, segOE=sOE,
            segD=segD, segE=segE,
            dv_blk=dv.reshape(NBLK, 128).T.copy(),
            dvsq_blk=(dv * dv).reshape(NBLK, 128).T.copy(),
            de_blk=de.reshape(EBLK, 128).T.copy()))
    return cores, (KAB, KC, KOD, KOE)


def _build(KAB, KC, KOD, KOE):
    import concourse.bass as bass
    import concourse.mybir as mybir
    from concourse import bacc, masks
    from concourse.tile import TileContext

    f32, bf16, i32 = mybir.dt.float32, mybir.dt.bfloat16, mybir.dt.int32
    nc = bacc.Bacc("TRN2", num_devices=W)
    T = lambda n, s, d=f32: nc.dram_tensor(n, s, d, kind="ExternalInput")
    xT = T("xT", [CONCAT, C_IN, NPC], bf16)
    W1 = T("W1", [CONCAT, C_IN, C_HID], bf16)
    b1c = T("b1c", [1, C], bf16)
    W2p = T("W2p", [C, CO], bf16)
    dv_blk = T("dv_blk", [128, NBLK]); dvsq_blk = T("dvsq_blk", [128, NBLK])
    de_blk = T("de_blk", [128, EBLK])
    idxAB = T("idxAB", [128, EBLK * KAB], i32); segAB = T("segAB", [128, EBLK * KAB], bf16)
    idxC = T("idxC", [128, NBLK * KC], i32); segC = T("segC", [128, NBLK * KC], bf16)
    idxOD = T("idxOD", [128, EBLK * KOD], i32); segOD = T("segOD", [128, EBLK * KOD], bf16)
    idxOE = T("idxOE", [128, NBLK * KOE], i32); segOE = T("segOE", [128, NBLK * KOE], bf16)
    segDd = T("segD", [128, NBLK_F * EBLK], bf16)
    segEd = T("segE", [128, EBLK_F * NBLK], bf16)
    iota_d = T("iota", [128, 128], bf16)
    out_own = nc.dram_tensor("out_own", [NPC, C_OUT_P], f32, kind="ExternalOutput")
    I = lambda n, s: nc.dram_tensor(n, s, bf16, kind="Internal")
    S = lambda n, s: nc.dram_tensor(n, s, bf16, kind="Internal", addr_space="Shared")
    y_own, y_full = I("y_own", [NPC, C]), S("y_full", [NP_, C])
    ef_own, ef_full = I("ef_own", [EPC, C]), S("ef_full", [EP_, C])
    y2_own, y2_full = I("y2_own", [NPC, CO]), S("y2_full", [NP_, CO])
    ef2_own, ef2_full = I("ef2_own", [EPC, CO]), S("ef2_full", [EP_, CO])
    RG = [list(range(W))]
    AG = lambda i, o: nc.gpsimd.collective_compute(
        "AllGather", mybir.AluOpType.bypass, replica_groups=RG, ins=[i[:]], outs=[o[:]])

    with TileContext(nc) as tc:
        with tc.tile_pool(name="const", bufs=1) as cp:
            w1_sb = cp.tile([128, CONCAT * 2 * C_HID], bf16)
            for k in range(CONCAT):
                for q in range(2):
                    nc.sync.dma_start(
                        w1_sb[:, (k * 2 + q) * C_HID:(k * 2 + q + 1) * C_HID],
                        W1[k, q * 128:(q + 1) * 128, :])
            w2_sb = cp.tile([128, 8 * CO], bf16)
            for f in range(8):
                nc.sync.dma_start(w2_sb[:, f * CO:(f + 1) * CO],
                                  W2p[f * 128:(f + 1) * 128, :])
            b1_sb = cp.tile([1, C], bf16); nc.sync.dma_start(b1_sb[:], b1c[:])
            ones_sb = cp.tile([1, 128], bf16); nc.vector.memset(ones_sb[:], 1.0)
            iota_sb = cp.tile([128, 128], bf16); nc.sync.dma_start(iota_sb[:], iota_d[:])
            ident = cp.tile([128, 128], f32); masks.make_identity(nc, ident[:])
            dv_sb = cp.tile([128, NBLK], f32); nc.sync.dma_start(dv_sb[:], dv_blk[:])
            dvsq_sb = cp.tile([128, NBLK], f32); nc.sync.dma_start(dvsq_sb[:], dvsq_blk[:])
            de_sb = cp.tile([128, EBLK], f32); nc.sync.dma_start(de_sb[:], de_blk[:])

            mm = lambda *a, **kw: nc.tensor.matmul(*a, skip_group_check=True, **kw)

            def seg_pass(K, idx_sb, seg_sb, src_full, elem, pool, ps_ap,
                         start_stream, stop_stream):
                """Indirect-gather + one-hot-matmul accumulation, one stream."""
                nbat = (K + BATCH - 1) // BATCH
                for s in range(nbat):
                    k0 = s * BATCH
                    nch = min(BATCH, K - k0)
                    oh = pool.tile([128, BATCH, 128], bf16, tag="oh")
                    nc.vector.tensor_tensor(
                        out=oh[:, :nch, :],
                        in0=iota_sb[:, None, :].broadcast_to([128, nch, 128]),
                        in1=seg_sb[:, k0:k0 + nch, None].broadcast_to([128, nch, 128]),
                        op=mybir.AluOpType.is_equal)
                    for j in range(nch):
                        g = pool.tile([128, elem], bf16, tag="g%d" % elem, bufs=4)
                        nc.gpsimd.indirect_dma_start(
                            out=g[:], out_offset=None, in_=src_full,
                            in_offset=bass.IndirectOffsetOnAxis(
                                ap=idx_sb[:, k0 + j:k0 + j + 1], axis=0))
                        first = start_stream and (s == 0 and j == 0)
                        last = stop_stream and (k0 + j == K - 1)
                        for h in range((elem + 511) // 512):
                            w_ = min(512, elem - h * 512)
                            mm(ps_ap[:, h * 512:h * 512 + w_],
                               lhsT=oh[:, j, :], rhs=g[:, h * 512:h * 512 + w_],
                               start=first, stop=last)

            # ---- phase A: y = dv * (x @ W1 + 1 b1) ----
            with nc.named_scope("phA"), \
                 tc.tile_pool(name="pa", bufs=3) as pa, \
                 tc.tile_pool(name="pap", bufs=2, space="PSUM") as pap:
                for b in range(NBLK):
                    ps = pap.tile([128, C], f32, tag="psA")
                    mm(ps[:, :512], lhsT=ones_sb[:, :], rhs=b1_sb[:, :512], start=True, stop=False)
                    mm(ps[:, 512:], lhsT=ones_sb[:, :], rhs=b1_sb[:, 512:], start=True, stop=False)
                    for k in range(CONCAT):
                        for q in range(2):
                            xt = pa.tile([128, 128], bf16, tag="xt")
                            nc.sync.dma_start(xt[:], xT[k, q * 128:(q + 1) * 128,
                                                        b * 128:(b + 1) * 128])
                            mm(ps[:, k * C_HID:(k + 1) * C_HID], lhsT=xt[:],
                               rhs=w1_sb[:, (k * 2 + q) * C_HID:(k * 2 + q + 1) * C_HID],
                               start=False, stop=(q == 1))
                    y_sb = pa.tile([128, C], bf16, tag="ysb")
                    nc.vector.tensor_tensor(
                        out=y_sb[:], in0=ps[:],
                        in1=dv_sb[:, b:b + 1].broadcast_to([128, C]),
                        op=mybir.AluOpType.mult)
                    nc.sync.dma_start(y_own[b * 128:(b + 1) * 128, :], y_sb[:])
            with nc.named_scope("AGy"):
                AG(y_own, y_full)

            with tc.tile_pool(name="bstream", bufs=1) as bs:
                iAB = bs.tile([128, EBLK * KAB], i32); nc.sync.dma_start(iAB[:], idxAB[:])
                sAB = bs.tile([128, EBLK * KAB], bf16); nc.sync.dma_start(sAB[:], segAB[:])
                iC = bs.tile([128, NBLK * KC], i32); nc.sync.dma_start(iC[:], idxC[:])
                sC = bs.tile([128, NBLK * KC], bf16); nc.sync.dma_start(sC[:], segC[:])

                # ---- phase B: ef = de * (H^T y) over own edges ----
                with nc.named_scope("phB"), \
                     tc.tile_pool(name="pb", bufs=2) as pb, \
                     tc.tile_pool(name="pbp", bufs=2, space="PSUM") as pbp:
                    for b in range(EBLK):
                        ps = pbp.tile([128, C], f32, tag="psB")
                        seg_pass(KAB, iAB[:, b * KAB:], sAB[:, b * KAB:],
                                 y_full[:], C, pb, ps, True, True)
                        ef_sb = pb.tile([128, C], bf16, tag="efsb")
                        nc.vector.tensor_tensor(
                            out=ef_sb[:], in0=ps[:],
                            in1=de_sb[:, b:b + 1].broadcast_to([128, C]),
                            op=mybir.AluOpType.mult)
                        nc.sync.dma_start(ef_own[b * 128:(b + 1) * 128, :], ef_sb[:])
                with nc.named_scope("AGef"):
                    AG(ef_own, ef_full)

                # ---- phase C: u = relu(H ef); y2 = dvsq * (u @ W2) ----
                with nc.named_scope("phC"), \
                     tc.tile_pool(name="pc", bufs=2) as pc, \
                     tc.tile_pool(name="pcp", bufs=2, space="PSUM") as pcp, \
                     tc.tile_pool(name="pct", bufs=1, space="PSUM") as pct:
                    for b in range(NBLK):
                        pz = pcp.tile([128, C], f32, tag="psC")
                        seg_pass(KC, iC[:, b * KC:], sC[:, b * KC:], ef_full[:],
                                 C, pc, pz, True, True)
                        u_sb = pc.tile([128, C], f32, tag="usb")
                        nc.scalar.activation(out=u_sb[:], in_=pz[:],
                                             func=mybir.ActivationFunctionType.Relu)
                        pt = pct.tile([128, C], f32, tag="ptC")
                        for f in range(8):
                            nc.tensor.transpose(pt[:, f * 128:(f + 1) * 128],
                                                u_sb[:, f * 128:(f + 1) * 128], ident[:])
                        ut_sb = pc.tile([128, C], bf16, tag="utsb")
                        nc.vector.tensor_copy(ut_sb[:], pt[:])
                        po = pct.tile([128, CO], f32, tag="poC")
                        for f in range(8):
                            mm(po[:], lhsT=ut_sb[:, f * 128:(f + 1) * 128],
                               rhs=w2_sb[:, f * CO:(f + 1) * CO],
                               start=(f == 0), stop=(f == 7))
                        y2_sb = pc.tile([128, CO], bf16, tag="y2sb")
                        nc.vector.tensor_tensor(
                            out=y2_sb[:], in0=po[:],
                            in1=dvsq_sb[:, b:b + 1].broadcast_to([128, CO]),
                            op=mybir.AluOpType.mult)
                        nc.sync.dma_start(y2_own[b * 128:(b + 1) * 128, :], y2_sb[:])
                with nc.named_scope("AGy2"):
                    AG(y2_own, y2_full)

            # ---- phase D: ef2 = de * (H^T y2), dense blocks + overflow ----
            with tc.tile_pool(name="dstream", bufs=1) as ds:
                sD = ds.tile([128, NBLK_F * EBLK], bf16); nc.sync.dma_start(sD[:], segDd[:])
                sE = ds.tile([128, EBLK_F * NBLK], bf16); nc.sync.dma_start(sE[:], segEd[:])
                iOD = ds.tile([128, EBLK * KOD], i32); nc.sync.dma_start(iOD[:], idxOD[:])
                sOD = ds.tile([128, EBLK * KOD], bf16); nc.sync.dma_start(sOD[:], segOD[:])
                iOE = ds.tile([128, NBLK * KOE], i32); nc.sync.dma_start(iOE[:], idxOE[:])
                sOE = ds.tile([128, NBLK * KOE], bf16); nc.sync.dma_start(sOE[:], segOE[:])

                with nc.named_scope("phD"), \
                     tc.tile_pool(name="pd", bufs=3) as pd, \
                     tc.tile_pool(name="pdo", bufs=2) as pdo, \
                     tc.tile_pool(name="pdp", bufs=1, space="PSUM") as pdp:
                    psD = pdp.tile([128, EBLK * CO], f32)
                    for nb in range(NBLK_F):
                        yt = pd.tile([128, CO], bf16, tag="ytD")
                        nc.sync.dma_start(yt[:], y2_full[nb * 128:(nb + 1) * 128, :])
                        ohs = pd.tile([128, EBLK, 128], bf16, tag="ohD")
                        nc.vector.tensor_tensor(
                            out=ohs[:],
                            in0=iota_sb[:, None, :].broadcast_to([128, EBLK, 128]),
                            in1=sD[:, nb * EBLK:(nb + 1) * EBLK, None]
                                .broadcast_to([128, EBLK, 128]),
                            op=mybir.AluOpType.is_equal)
                        for eb in range(EBLK):
                            mm(psD[:, eb * CO:(eb + 1) * CO],
                               lhsT=ohs[:, eb, :], rhs=yt[:],
                               start=(nb == 0), stop=False)
                    for eb in range(EBLK):
                        seg_pass(KOD, iOD[:, eb * KOD:], sOD[:, eb * KOD:],
                                 y2_full[:], CO, pdo,
                                 psD[:, eb * CO:(eb + 1) * CO], False, True)
                        e2_sb = pdo.tile([128, CO], bf16, tag="e2sb")
                        nc.vector.tensor_tensor(
                            out=e2_sb[:], in0=psD[:, eb * CO:(eb + 1) * CO],
                            in1=de_sb[:, eb:eb + 1].broadcast_to([128, CO]),
                            op=mybir.AluOpType.mult)
                        nc.sync.dma_start(ef2_own[eb * 128:(eb + 1) * 128, :], e2_sb[:])
                with nc.named_scope("AGef2"):
                    AG(ef2_own, ef2_full)

                # ---- phase E: res = dv * (H ef2), dense blocks + overflow ----
                with nc.named_scope("phE"), \
                     tc.tile_pool(name="pe", bufs=3) as pe_, \
                     tc.tile_pool(name="peo", bufs=2) as peo, \
                     tc.tile_pool(name="pep", bufs=1, space="PSUM") as pep:
                    NSW = max(s1 - s0 for s0, s1 in ESW)
                    for s0, s1 in ESW:
                        nsw = s1 - s0
                        psE = pep.tile([128, NSW * CO], f32, tag="psE",
                                       name="psE%d" % s0)
                        for eb in range(EBLK_F):
                            et = pe_.tile([128, CO], bf16, tag="etE")
                            nc.sync.dma_start(et[:], ef2_full[eb * 128:(eb + 1) * 128, :])
                            ohs = pe_.tile([128, NSW, 128], bf16, tag="ohE")
                            nc.vector.tensor_tensor(
                                out=ohs[:, :nsw, :],
                                in0=iota_sb[:, None, :].broadcast_to([128, nsw, 128]),
                                in1=sE[:, eb * NBLK + s0:eb * NBLK + s1, None]
                                    .broadcast_to([128, nsw, 128]),
                                op=mybir.AluOpType.is_equal)
                            for k in range(nsw):
                                mm(psE[:, k * CO:(k + 1) * CO],
                                   lhsT=ohs[:, k, :], rhs=et[:],
                                   start=(eb == 0), stop=False)
                        for k in range(nsw):
                            nb = s0 + k
                            seg_pass(KOE, iOE[:, nb * KOE:], sOE[:, nb * KOE:],
                                     ef2_full[:], CO, peo,
                                     psE[:, k * CO:(k + 1) * CO], False, True)
                            o_sb = peo.tile([128, C_OUT_P], f32, tag="osb")
                            nc.vector.tensor_tensor(
                                out=o_sb[:], in0=psE[:, k * CO:k * CO + C_OUT_P],
                                in1=dv_sb[:, nb:nb + 1].broadcast_to([128, C_OUT_P]),
                                op=mybir.AluOpType.mult)
                            nc.sync.dma_start(out_own[nb * 128:(nb + 1) * 128, :], o_sb[:])
    nc.finalize()
    return nc


_CACHE = {}


def kernel(x_list, W1, b1, W2, b2, node_idx, edge_idx, n_edges, _trace=False):
    import ml_dtypes
    from concourse import bass_utils
    bfloat16 = ml_dtypes.bfloat16
    x_list = np.asarray(x_list, np.float32); W1 = np.asarray(W1, np.float32)
    b1 = np.asarray(b1, np.float32); W2 = np.asarray(W2, np.float32)
    b2 = np.asarray(b2, np.float32)
    node_idx = np.asarray(node_idx, np.int32); edge_idx = np.asarray(edge_idx, np.int32)

    dv = np.bincount(node_idx, minlength=N).astype(np.float32)
    de = np.bincount(edge_idx, minlength=E).astype(np.float32)
    dv_is = np.where(dv > 0, 1.0 / np.sqrt(np.maximum(dv, 1.0)), 0.0).astype(np.float32)
    de_inv = np.where(de > 0, 1.0 / np.maximum(de, 1.0), 0.0).astype(np.float32)
    ef_t = np.bincount(edge_idx, weights=dv_is[node_idx], minlength=E) * de_inv
    s1 = dv_is * np.bincount(node_idx, weights=ef_t[edge_idx], minlength=N)

    cores, Ks = _prep(node_idx, edge_idx, dv_is, de_inv)
    if Ks not in _CACHE:
        _CACHE[Ks] = _build(*Ks)
    nc = _CACHE[Ks]

    W2p = np.zeros((C, CO), np.float32)
    W2p[:, :C_OUT] = W2
    iota_np = np.tile(np.arange(128, dtype=np.float32), (128, 1))
    in_maps = []
    for c in range(W):
        xTc = np.zeros((CONCAT, C_IN, NPC), np.float32)
        xTc[:, :, :NPC_R] = x_list[:, c * NPC_R:(c + 1) * NPC_R, :].transpose(0, 2, 1)
        cd = dict(cores[c])
        for k in ("segAB", "segC", "segOD", "segOE", "segD", "segE"):
            cd[k] = cd[k].astype(bfloat16)
        m = dict(xT=xTc.astype(bfloat16), W1=W1.astype(bfloat16),
                 b1c=b1.reshape(1, C).astype(bfloat16), W2p=W2p.astype(bfloat16),
                 iota=iota_np.astype(bfloat16), **cd)
        in_maps.append(m)
    try:
        res = bass_utils.run_bass_kernel_spmd(nc, in_maps, core_ids=list(range(W)),
                                              trace=_trace)
    except ModuleNotFoundError:
        res = bass_utils.run_bass_kernel_spmd(nc, in_maps, core_ids=list(range(W)),
                                              trace=False)
    out = np.empty((N, C_OUT), np.float32)
    for c in range(W):
        out[c * NPC_R:(c + 1) * NPC_R] = res.results[c]["out_own"][:NPC_R, :C_OUT]
    out += np.outer(s1, b2)
    kernel._last = res
    return out


# revision 6
# speedup vs baseline: 1.0743x; 1.0743x over previous
"""LAHGCN hypergraph-conv kernel for 8 Trainium2 NeuronCores — v5.

v5 vs v4a:
  - All per-incidence gathers use indirect_dma_start (hardware dynamic DGE,
    int32 per-partition row offsets) instead of dma_gather (software Q7
    descriptor generation at ~8ns/row, which was the B/C bottleneck).
  - int32 offsets remove the int16 row-index limit: B and phase-D-overflow
    lo/hi stream splits are merged into single streams.
  - Phases D/E stay dense-block (no per-incidence work at all).
"""
import numpy as np

N, E, NNZ = 50000, 20000, 1600000
CONCAT, C_IN, C_HID = 4, 256, 256
C = CONCAT * C_HID            # 1024
C_OUT, C_OUT_P = 40, 64
CO = 64                       # padded col count for smooth-2 tables (bf16)
W = 8
NPC_R, EPC_R = N // W, E // W           # 6250, 2500 real per core
NBLK, EBLK = 49, 20
NPC, EPC = NBLK * 128, EBLK * 128       # 6272, 2560 padded per core
NP_, EP_ = W * NPC, W * EPC             # 50176, 20480
NBLK_F, EBLK_F = NP_ // 128, EP_ // 128  # 392, 160 full-table blocks
BATCH = 16                              # one-hot chunks per is_equal build
ESW = [(0, 25), (25, 49)]               # phase E psum sweeps over node blocks


def _streams32(rows, segpos, nblk, K):
    """int32 idx table [128, nblk*K] + seg table [128, nblk*K].
    Entry i of block b -> partition i%128, column b*K + i//128."""
    idx = np.zeros((128, nblk * K), np.int32)
    seg = np.full((128, nblk * K), -1.0, np.float32)
    for b in range(nblk):
        r, p = rows[b], segpos[b]
        n = len(r)
        assert n <= K * 128
        part = np.arange(n) % 128
        col = b * K + np.arange(n) // 128
        idx[part, col] = r.astype(np.int32)
        seg[part, col] = p.astype(np.float32)
    return idx, seg


def _split_rank(src_loc, pair, pos, npairs):
    """Rank entries within (src row mod 128, pair); k=0 goes to the dense seg
    table [128, npairs], k>0 to overflow lists."""
    order = np.lexsort((pos, pair * 128 + src_loc))
    sl, pr, po = src_loc[order], pair[order], pos[order]
    key = pr * 128 + sl
    first = np.ones(len(key), bool)
    first[1:] = key[1:] != key[:-1]
    seg = np.full((128, npairs), -1.0, np.float32)
    seg[sl[first], pr[first]] = po[first].astype(np.float32)
    return seg, order[~first]


def _prep(node_idx, edge_idx, dv_is, de_inv):
    """All host-side index prep. Returns per-core dicts of arrays + loop Ks."""
    nrow = (node_idx // NPC_R) * NPC + node_idx % NPC_R    # node -> y row
    erow = (edge_idx // EPC_R) * EPC + edge_idx % EPC_R    # edge -> ef row
    p1 = np.argsort(edge_idx, kind="stable")
    e1, n1 = edge_idx[p1], nrow[p1]
    p2 = np.argsort(node_idx, kind="stable")
    n2, e2 = node_idx[p2], erow[p2]
    per = []
    for c in range(W):
        m1 = (e1 >= c * EPC_R) & (e1 < (c + 1) * EPC_R)
        el = e1[m1] - c * EPC_R        # local edge row (edge-sorted)
        nr = n1[m1]                    # full node row (y table)
        ab_rows, ab_pos = [], []
        for b in range(EBLK):
            mb = (el >= b * 128) & (el < (b + 1) * 128)
            ab_rows.append(nr[mb]); ab_pos.append(el[mb] - b * 128)
        m2 = (n2 >= c * NPC_R) & (n2 < (c + 1) * NPC_R)
        nl = n2[m2] - c * NPC_R        # local node row (node-sorted)
        er = e2[m2]                    # full edge row (ef table)
        c_rows, c_pos = [], []
        for b in range(NBLK):
            mb = (nl >= b * 128) & (nl < (b + 1) * 128)
            c_rows.append(er[mb]); c_pos.append(nl[mb] - b * 128)

        # phase D dense/overflow (edge-sorted): pair (nb 0..391, eb 0..19)
        nbD, ebD = nr // 128, el // 128
        pairD = nbD * EBLK + ebD
        segD, ovD = _split_rank(nr % 128, pairD, el % 128, NBLK_F * EBLK)
        ovD_r, ovD_eb, ovD_pos = nr[ovD], ebD[ovD], (el % 128)[ovD]
        d_rows, d_pos = [], []
        for b in range(EBLK):
            mb = ovD_eb == b
            d_rows.append(ovD_r[mb]); d_pos.append(ovD_pos[mb])

        # phase E dense/overflow (node-sorted): pair (eb 0..159, nb 0..48)
        ebE, nbE = er // 128, nl // 128
        pairE = ebE * NBLK + nbE
        segE, ovE = _split_rank(er % 128, pairE, nl % 128, EBLK_F * NBLK)
        ovE_r, ovE_nb, ovE_pos = er[ovE], nbE[ovE], (nl % 128)[ovE]
        e_rows, e_pos = [], []
        for b in range(NBLK):
            mb = ovE_nb == b
            e_rows.append(ovE_r[mb]); e_pos.append(ovE_pos[mb])

        per.append((ab_rows, ab_pos, c_rows, c_pos,
                    segD, (d_rows, d_pos), segE, (e_rows, e_pos)))

    kmax = lambda lists: max(1, max(max(((len(r) + 127) // 128 for r in p),
                                        default=0) for p in lists))
    KAB = kmax([p[0] for p in per])
    KC = kmax([p[2] for p in per])
    KOD = kmax([p[5][0] for p in per])
    KOE = kmax([p[7][0] for p in per])
    cores = []
    for c in range(W):
        (ab_rows, ab_pos, c_rows, c_pos,
         segD, (d_r, d_p), segE, (e_r, e_p)) = per[c]
        iAB, sAB = _streams32(ab_rows, ab_pos, EBLK, KAB)
        iC, sC = _streams32(c_rows, c_pos, NBLK, KC)
        iOD, sOD = _streams32(d_r, d_p, EBLK, KOD)
        iOE, sOE = _streams32(e_r, e_p, NBLK, KOE)
        dv = np.zeros(NPC, np.float32)
        dv[:NPC_R] = dv_is[c * NPC_R:(c + 1) * NPC_R]
        de = np.zeros(EPC, np.float32)
        de[:EPC_R] = de_inv[c * EPC_R:(c + 1) * EPC_R]
        cores.append(dict(
            idxAB=iAB, segAB=sAB, idxC=iC, segC=sC,
            idxOD=iOD, segOD=sOD, idxOE=iOE, segOE=sOE,
            segD=segD, segE=segE,
            dv_row=dv.reshape(1, NPC).copy(),
            dv_blk=dv.reshape(NBLK, 128).T.copy(),
            dvsq_blk=(dv * dv).reshape(NBLK, 128).T.copy(),
            de_blk=de.reshape(EBLK, 128).T.copy()))
    return cores, (KAB, KC, KOD, KOE)


def _build(KAB, KC, KOD, KOE):
    import concourse.bass as bass
    import concourse.mybir as mybir
    from concourse import bacc, masks
    from concourse.tile import TileContext

    f32, bf16, i32 = mybir.dt.float32, mybir.dt.bfloat16, mybir.dt.int32
    nc = bacc.Bacc("TRN2", num_devices=W)
    T = lambda n, s, d=f32: nc.dram_tensor(n, s, d, kind="ExternalInput")
    xT = T("xT", [CONCAT, C_IN, NPC], bf16)
    W1 = T("W1", [CONCAT, C_IN, C_HID], bf16)
    b1c = T("b1c", [1, C], bf16)
    W2p = T("W2p", [C, CO], bf16)
    dv_blk = T("dv_blk", [128, NBLK]); dvsq_blk = T("dvsq_blk", [128, NBLK])
    dv_row = T("dv_row", [1, NPC])
    de_blk = T("de_blk", [128, EBLK])
    idxAB = T("idxAB", [128, EBLK * KAB], i32); segAB = T("segAB", [128, EBLK * KAB], bf16)
    idxC = T("idxC", [128, NBLK * KC], i32); segC = T("segC", [128, NBLK * KC], bf16)
    idxOD = T("idxOD", [128, EBLK * KOD], i32); segOD = T("segOD", [128, EBLK * KOD], bf16)
    idxOE = T("idxOE", [128, NBLK * KOE], i32); segOE = T("segOE", [128, NBLK * KOE], bf16)
    segDd = T("segD", [128, NBLK_F * EBLK], bf16)
    segEd = T("segE", [128, EBLK_F * NBLK], bf16)
    iota_d = T("iota", [128, 128], bf16)
    out_own = nc.dram_tensor("out_own", [C_OUT_P, NPC], f32, kind="ExternalOutput")
    I = lambda n, s: nc.dram_tensor(n, s, bf16, kind="Internal")
    S = lambda n, s: nc.dram_tensor(n, s, bf16, kind="Internal", addr_space="Shared")
    y_own, y_full = I("y_own", [NPC, C]), S("y_full", [NP_, C])
    ef_own, ef_full = I("ef_own", [EPC, C]), S("ef_full", [EP_, C])
    y2_own, y2_full = I("y2_own", [NPC, CO]), S("y2_full", [NP_, CO])
    ef2_own, ef2_full = I("ef2_own", [EPC, CO]), S("ef2_full", [EP_, CO])
    RG = [list(range(W))]
    AG = lambda i, o: nc.gpsimd.collective_compute(
        "AllGather", mybir.AluOpType.bypass, replica_groups=RG, ins=[i[:]], outs=[o[:]])

    with TileContext(nc) as tc:
        with tc.tile_pool(name="const", bufs=1) as cp:
            w1_sb = cp.tile([128, CONCAT * 2 * C_HID], bf16)
            for k in range(CONCAT):
                for q in range(2):
                    nc.sync.dma_start(
                        w1_sb[:, (k * 2 + q) * C_HID:(k * 2 + q + 1) * C_HID],
                        W1[k, q * 128:(q + 1) * 128, :])
            w2_sb = cp.tile([128, 8 * CO], bf16)
            for f in range(8):
                nc.sync.dma_start(w2_sb[:, f * CO:(f + 1) * CO],
                                  W2p[f * 128:(f + 1) * 128, :])
            b1_sb = cp.tile([1, C], bf16); nc.sync.dma_start(b1_sb[:], b1c[:])
            ones_sb = cp.tile([1, 128], bf16); nc.vector.memset(ones_sb[:], 1.0)
            iota_sb = cp.tile([128, 128], bf16); nc.sync.dma_start(iota_sb[:], iota_d[:])
            ident = cp.tile([128, 128], f32); masks.make_identity(nc, ident[:])
            dv_sb = cp.tile([128, NBLK], f32); nc.sync.dma_start(dv_sb[:], dv_blk[:])
            dvsq_sb = cp.tile([128, NBLK], f32); nc.sync.dma_start(dvsq_sb[:], dvsq_blk[:])
            de_sb = cp.tile([128, EBLK], f32); nc.sync.dma_start(de_sb[:], de_blk[:])
            dvr_sb = cp.tile([1, NPC], f32); nc.sync.dma_start(dvr_sb[:], dv_row[:])

            mm = lambda *a, **kw: nc.tensor.matmul(*a, skip_group_check=True, **kw)

            def seg_passT(K, idx_sb, seg_sb, src_full, elem, pool, ps_ap,
                          start_stream, stop_stream):
                """Like seg_pass but accumulates transposed: ps[c, seg]."""
                nbat = (K + BATCH - 1) // BATCH
                for s in range(nbat):
                    k0 = s * BATCH
                    nch = min(BATCH, K - k0)
                    oh = pool.tile([128, BATCH, 128], bf16, tag="ohT")
                    nc.vector.tensor_tensor(
                        out=oh[:, :nch, :],
                        in0=iota_sb[:, None, :].broadcast_to([128, nch, 128]),
                        in1=seg_sb[:, k0:k0 + nch, None].broadcast_to([128, nch, 128]),
                        op=mybir.AluOpType.is_equal)
                    for j in range(nch):
                        g = pool.tile([128, elem], bf16, tag="gT%d" % elem, bufs=6)
                        nc.gpsimd.indirect_dma_start(
                            out=g[:], out_offset=None, in_=src_full,
                            in_offset=bass.IndirectOffsetOnAxis(
                                ap=idx_sb[:, k0 + j:k0 + j + 1], axis=0))
                        first = start_stream and (s == 0 and j == 0)
                        last = stop_stream and (k0 + j == K - 1)
                        mm(ps_ap[:, :], lhsT=g[:, :C_OUT_P], rhs=oh[:, j, :],
                           start=first, stop=last)

            def seg_pass(K, idx_sb, seg_sb, src_full, elem, pool, ps_ap,
                         start_stream, stop_stream):
                """Indirect-gather + one-hot-matmul accumulation, one stream."""
                nbat = (K + BATCH - 1) // BATCH
                for s in range(nbat):
                    k0 = s * BATCH
                    nch = min(BATCH, K - k0)
                    oh = pool.tile([128, BATCH, 128], bf16, tag="oh")
                    nc.vector.tensor_tensor(
                        out=oh[:, :nch, :],
                        in0=iota_sb[:, None, :].broadcast_to([128, nch, 128]),
                        in1=seg_sb[:, k0:k0 + nch, None].broadcast_to([128, nch, 128]),
                        op=mybir.AluOpType.is_equal)
                    for j in range(nch):
                        g = pool.tile([128, elem], bf16, tag="g%d" % elem, bufs=6)
                        nc.gpsimd.indirect_dma_start(
                            out=g[:], out_offset=None, in_=src_full,
                            in_offset=bass.IndirectOffsetOnAxis(
                                ap=idx_sb[:, k0 + j:k0 + j + 1], axis=0))
                        first = start_stream and (s == 0 and j == 0)
                        last = stop_stream and (k0 + j == K - 1)
                        for h in range((elem + 511) // 512):
                            w_ = min(512, elem - h * 512)
                            mm(ps_ap[:, h * 512:h * 512 + w_],
                               lhsT=oh[:, j, :], rhs=g[:, h * 512:h * 512 + w_],
                               start=first, stop=last)

            # ---- phase A: y = dv * (x @ W1 + 1 b1) ----
            with nc.named_scope("phA"), \
                 tc.tile_pool(name="pa", bufs=3) as pa, \
                 tc.tile_pool(name="pax", bufs=1) as pax, \
                 tc.tile_pool(name="pap", bufs=2, space="PSUM") as pap:
                xsb = pax.tile([128, CONCAT * 2 * NPC], bf16)
                for k in range(CONCAT):
                    for q in range(2):
                        nc.sync.dma_start(
                            xsb[:, (k * 2 + q) * NPC:(k * 2 + q + 1) * NPC],
                            xT[k, q * 128:(q + 1) * 128, :])
                for b in range(NBLK):
                    ps = pap.tile([128, C], f32, tag="psA")
                    mm(ps[:, :512], lhsT=ones_sb[:, :], rhs=b1_sb[:, :512], start=True, stop=False)
                    mm(ps[:, 512:], lhsT=ones_sb[:, :], rhs=b1_sb[:, 512:], start=True, stop=False)
                    for k in range(CONCAT):
                        for q in range(2):
                            mm(ps[:, k * C_HID:(k + 1) * C_HID],
                               lhsT=xsb[:, (k * 2 + q) * NPC + b * 128:
                                        (k * 2 + q) * NPC + (b + 1) * 128],
                               rhs=w1_sb[:, (k * 2 + q) * C_HID:(k * 2 + q + 1) * C_HID],
                               start=False, stop=(q == 1))
                    y_sb = pa.tile([128, C], bf16, tag="ysb")
                    nc.vector.tensor_tensor(
                        out=y_sb[:], in0=ps[:],
                        in1=dv_sb[:, b:b + 1].broadcast_to([128, C]),
                        op=mybir.AluOpType.mult)
                    nc.sync.dma_start(y_own[b * 128:(b + 1) * 128, :], y_sb[:])
            with nc.named_scope("AGy"):
                AG(y_own, y_full)

            with tc.tile_pool(name="bstream", bufs=1) as bs:
                iAB = bs.tile([128, EBLK * KAB], i32); nc.sync.dma_start(iAB[:], idxAB[:])
                sAB = bs.tile([128, EBLK * KAB], bf16); nc.sync.dma_start(sAB[:], segAB[:])
                iC = bs.tile([128, NBLK * KC], i32); nc.sync.dma_start(iC[:], idxC[:])
                sC = bs.tile([128, NBLK * KC], bf16); nc.sync.dma_start(sC[:], segC[:])

                # ---- phase B: ef = de * (H^T y) over own edges ----
                with nc.named_scope("phB"), \
                     tc.tile_pool(name="pb", bufs=3) as pb, \
                     tc.tile_pool(name="pbp", bufs=2, space="PSUM") as pbp:
                    for b in range(EBLK):
                        ps = pbp.tile([128, C], f32, tag="psB")
                        seg_pass(KAB, iAB[:, b * KAB:], sAB[:, b * KAB:],
                                 y_full[:], C, pb, ps, True, True)
                        ef_sb = pb.tile([128, C], bf16, tag="efsb")
                        nc.vector.tensor_tensor(
                            out=ef_sb[:], in0=ps[:],
                            in1=de_sb[:, b:b + 1].broadcast_to([128, C]),
                            op=mybir.AluOpType.mult)
                        nc.sync.dma_start(ef_own[b * 128:(b + 1) * 128, :], ef_sb[:])
                with nc.named_scope("AGef"):
                    AG(ef_own, ef_full)

                # ---- phase C: u = relu(H ef); y2 = dvsq * (u @ W2) ----
                with nc.named_scope("phC"), \
                     tc.tile_pool(name="pc", bufs=3) as pc, \
                     tc.tile_pool(name="pcp", bufs=2, space="PSUM") as pcp, \
                     tc.tile_pool(name="pct", bufs=1, space="PSUM") as pct:
                    for b in range(NBLK):
                        pz = pcp.tile([128, C], f32, tag="psC")
                        seg_pass(KC, iC[:, b * KC:], sC[:, b * KC:], ef_full[:],
                                 C, pc, pz, True, True)
                        u_sb = pc.tile([128, C], f32, tag="usb")
                        nc.scalar.activation(out=u_sb[:], in_=pz[:],
                                             func=mybir.ActivationFunctionType.Relu)
                        pt = pct.tile([128, C], f32, tag="ptC")
                        for f in range(8):
                            nc.tensor.transpose(pt[:, f * 128:(f + 1) * 128],
                                                u_sb[:, f * 128:(f + 1) * 128], ident[:])
                        ut_sb = pc.tile([128, C], bf16, tag="utsb")
                        nc.vector.tensor_copy(ut_sb[:], pt[:])
                        po = pct.tile([128, CO], f32, tag="poC")
                        for f in range(8):
                            mm(po[:], lhsT=ut_sb[:, f * 128:(f + 1) * 128],
                               rhs=w2_sb[:, f * CO:(f + 1) * CO],
                               start=(f == 0), stop=(f == 7))
                        y2_sb = pc.tile([128, CO], bf16, tag="y2sb")
                        nc.vector.tensor_tensor(
                            out=y2_sb[:], in0=po[:],
                            in1=dvsq_sb[:, b:b + 1].broadcast_to([128, CO]),
                            op=mybir.AluOpType.mult)
                        nc.sync.dma_start(y2_own[b * 128:(b + 1) * 128, :], y2_sb[:])
                with nc.named_scope("AGy2"):
                    AG(y2_own, y2_full)

            # ---- phase D: ef2 = de * (H^T y2), dense blocks + overflow ----
            with tc.tile_pool(name="dstream", bufs=1) as ds:
                sD = ds.tile([128, NBLK_F * EBLK], bf16); nc.sync.dma_start(sD[:], segDd[:])
                sE = ds.tile([128, EBLK_F * NBLK], bf16); nc.sync.dma_start(sE[:], segEd[:])
                iOD = ds.tile([128, EBLK * KOD], i32); nc.sync.dma_start(iOD[:], idxOD[:])
                sOD = ds.tile([128, EBLK * KOD], bf16); nc.sync.dma_start(sOD[:], segOD[:])
                iOE = ds.tile([128, NBLK * KOE], i32); nc.sync.dma_start(iOE[:], idxOE[:])
                sOE = ds.tile([128, NBLK * KOE], bf16); nc.sync.dma_start(sOE[:], segOE[:])

                with nc.named_scope("phD"), \
                     tc.tile_pool(name="pd", bufs=3) as pd, \
                     tc.tile_pool(name="pdo", bufs=2) as pdo, \
                     tc.tile_pool(name="pdp", bufs=1, space="PSUM") as pdp:
                    psDT = pdp.tile([C_OUT_P, EBLK * 128], f32)
                    for nb in range(NBLK_F):
                        yt = pd.tile([128, CO], bf16, tag="ytD")
                        nc.sync.dma_start(yt[:], y2_full[nb * 128:(nb + 1) * 128, :])
                        ohs = pd.tile([128, EBLK, 128], bf16, tag="ohD")
                        nc.vector.tensor_tensor(
                            out=ohs[:],
                            in0=iota_sb[:, None, :].broadcast_to([128, EBLK, 128]),
                            in1=sD[:, nb * EBLK:(nb + 1) * EBLK, None]
                                .broadcast_to([128, EBLK, 128]),
                            op=mybir.AluOpType.is_equal)
                        for j in range(EBLK // 4):
                            mm(psDT[:, j * 512:(j + 1) * 512],
                               lhsT=yt[:], rhs=ohs[:, j * 4:(j + 1) * 4, :],
                               start=(nb == 0), stop=False)
                    for eb in range(EBLK):
                        seg_passT(KOD, iOD[:, eb * KOD:], sOD[:, eb * KOD:],
                                  y2_full[:], CO, pdo,
                                  psDT[:, eb * 128:(eb + 1) * 128], False, True)
                        # psDT slice [64c, 128e] -> transpose back to [128e, 64c]
                        tT_sb = pdo.tile([C_OUT_P, 128], f32, tag="tTsb")
                        nc.vector.tensor_copy(tT_sb[:], psDT[:, eb * 128:(eb + 1) * 128])
                        ptr = pdp.tile([128, C_OUT_P], f32, tag="ptr")
                        nc.tensor.transpose(ptr[:], tT_sb[:], ident[:C_OUT_P, :C_OUT_P])
                        e2_sb = pdo.tile([128, CO], bf16, tag="e2sb")
                        nc.vector.tensor_tensor(
                            out=e2_sb[:], in0=ptr[:],
                            in1=de_sb[:, eb:eb + 1].broadcast_to([128, CO]),
                            op=mybir.AluOpType.mult)
                        nc.sync.dma_start(ef2_own[eb * 128:(eb + 1) * 128, :], e2_sb[:])
                with nc.named_scope("AGef2"):
                    AG(ef2_own, ef2_full)

                # ---- phase E: res = dv * (H ef2), dense blocks + overflow ----
                with nc.named_scope("phE"), \
                     tc.tile_pool(name="pe", bufs=3) as pe_, \
                     tc.tile_pool(name="peo", bufs=2) as peo, \
                     tc.tile_pool(name="pep", bufs=1, space="PSUM") as pep:
                    NSW = max(s1 - s0 for s0, s1 in ESW)
                    for s0, s1 in ESW:
                        nsw = s1 - s0
                        psET = pep.tile([C_OUT_P, NSW * 128], f32, tag="psET",
                                        name="psET%d" % s0)
                        for eb in range(EBLK_F):
                            et = pe_.tile([128, CO], bf16, tag="etE")
                            nc.sync.dma_start(et[:], ef2_full[eb * 128:(eb + 1) * 128, :])
                            ohs = pe_.tile([128, NSW, 128], bf16, tag="ohE")
                            nc.vector.tensor_tensor(
                                out=ohs[:, :nsw, :],
                                in0=iota_sb[:, None, :].broadcast_to([128, nsw, 128]),
                                in1=sE[:, eb * NBLK + s0:eb * NBLK + s1, None]
                                    .broadcast_to([128, nsw, 128]),
                                op=mybir.AluOpType.is_equal)
                            for j in range((nsw + 3) // 4):
                                b0, b1 = j * 4, min((j + 1) * 4, nsw)
                                mm(psET[:, b0 * 128:b1 * 128],
                                   lhsT=et[:], rhs=ohs[:, b0:b1, :],
                                   start=(eb == 0), stop=False)
                        for k in range(nsw):
                            nb = s0 + k
                            seg_passT(KOE, iOE[:, nb * KOE:], sOE[:, nb * KOE:],
                                      ef2_full[:], CO, peo,
                                      psET[:, k * 128:(k + 1) * 128], False, True)
                            o_sb = peo.tile([C_OUT_P, 128], f32, tag="osb")
                            nc.vector.tensor_copy(o_sb[:], psET[:, k * 128:(k + 1) * 128])
                            nc.sync.dma_start(out_own[:, nb * 128:(nb + 1) * 128], o_sb[:])
    nc.finalize()
    return nc


_CACHE = {}


def kernel(x_list, W1, b1, W2, b2, node_idx, edge_idx, n_edges, _trace=False):
    import ml_dtypes
    from concourse import bass_utils
    bfloat16 = ml_dtypes.bfloat16
    x_list = np.asarray(x_list, np.float32); W1 = np.asarray(W1, np.float32)
    b1 = np.asarray(b1, np.float32); W2 = np.asarray(W2, np.float32)
    b2 = np.asarray(b2, np.float32)
    node_idx = np.asarray(node_idx, np.int32); edge_idx = np.asarray(edge_idx, np.int32)

    dv = np.bincount(node_idx, minlength=N).astype(np.float32)
    de = np.bincount(edge_idx, minlength=E).astype(np.float32)
    dv_is = np.where(dv > 0, 1.0 / np.sqrt(np.maximum(dv, 1.0)), 0.0).astype(np.float32)
    de_inv = np.where(de > 0, 1.0 / np.maximum(de, 1.0), 0.0).astype(np.float32)
    ef_t = np.bincount(edge_idx, weights=dv_is[node_idx], minlength=E) * de_inv
    s1 = dv_is * np.bincount(node_idx, weights=ef_t[edge_idx], minlength=N)

    cores, Ks = _prep(node_idx, edge_idx, dv_is, de_inv)
    if Ks not in _CACHE:
        _CACHE[Ks] = _build(*Ks)
    nc = _CACHE[Ks]

    W2p = np.zeros((C, CO), np.float32)
    W2p[:, :C_OUT] = W2
    iota_np = np.tile(np.arange(128, dtype=np.float32), (128, 1))
    in_maps = []
    for c in range(W):
        xTc = np.zeros((CONCAT, C_IN, NPC), np.float32)
        xTc[:, :, :NPC_R] = x_list[:, c * NPC_R:(c + 1) * NPC_R, :].transpose(0, 2, 1)
        cd = dict(cores[c])
        for k in ("segAB", "segC", "segOD", "segOE", "segD", "segE"):
            cd[k] = cd[k].astype(bfloat16)
        m = dict(xT=xTc.astype(bfloat16), W1=W1.astype(bfloat16),
                 b1c=b1.reshape(1, C).astype(bfloat16), W2p=W2p.astype(bfloat16),
                 iota=iota_np.astype(bfloat16), **cd)
        in_maps.append(m)
    try:
        res = bass_utils.run_bass_kernel_spmd(nc, in_maps, core_ids=list(range(W)),
                                              trace=_trace)
    except ModuleNotFoundError:
        res = bass_utils.run_bass_kernel_spmd(nc, in_maps, core_ids=list(range(W)),
                                              trace=False)
    out = np.empty((N, C_OUT), np.float32)
    for c in range(W):
        out[c * NPC_R:(c + 1) * NPC_R] = res.results[c]["out_own"][:C_OUT, :NPC_R].T \
            * dv_is[c * NPC_R:(c + 1) * NPC_R, None]
    out += np.outer(s1, b2)
    kernel._last = res
    return out
